# revision 23
# baseline (speedup 1.0000x reference)
"""Capacity-MoE Trainium2 kernel (8 NeuronCores, expert-parallel).

Contract: kernel(**inputs) takes the FULL inputs of reference.setup_inputs()
and returns the FULL [B, D] float32 output.

Strategy
--------
Host: replicate the reference's capacity-aware routing (a plain cumsum over
the one-hot routes — no feedback loop), build per-expert accepted-token
lists, and shard expert e's tokens (<= capacity) to core e.  Tokens whose
every route overflowed ("dropped") are sharded contiguously across all 8
cores for the fallback MLP.  Device (per core): two dense 2-layer MLP
streams — the core's expert MLP over its [T_pad] gathered tokens and the
fallback MLP over its dropped-token chunk — all activations kept transposed
[D, T] so no on-device transposes are needed.  Host: gather per-assignment
outputs, average by accept count, and patch dropped rows with the fallback.

Matmuls run as float32r (TF32-like reduced-precision fp32, ~1.5e-4 relative
error per matmul, 4x the throughput of true fp32 on the PE array) with fp32
PSUM accumulation.
"""

import os
import sys

for _p in ("/opt/trn_rl_repo",):
    if _p not in sys.path and os.path.isdir(_p):
        sys.path.append(_p)

import numpy as np

import concourse.bass as bass
import concourse.tile as tile
from concourse import mybir
from concourse.bass_utils import run_bass_kernel_spmd

F32 = mybir.dt.float32
DT = mybir.dt.float32r  # matmul operand dtype

D = 1024
NCORES = 8
KCH = 8  # contraction chunks of 128 (D / 128)


# ---------------------------------------------------------------------------
# walrus in this environment rejects instructions with >1 sync wait; split
# extra waits onto same-engine NoOps inserted directly before the offender.
def _split_multi_waits(nc):
    ctr = 0
    for f in nc.m.functions:
        for bb in f.blocks:
            il = bb.instructions
            i = 0
            while i < len(il):
                inst = il[i]
                si = inst.sync_info
                if si is None or si.on_wait is None or len(si.on_wait) <= 1:
                    i += 1
                    continue
                waits = list(si.on_wait)
                for w in waits[:-1]:
                    ctr += 1
                    nop = mybir.InstNoOp(name=f"waitsplit-{ctr}")
                    nop.engine = inst.engine
                    nop.sync_info = mybir.SyncInfo(on_wait=[w], on_update=[])
                    il.insert(i, nop)
                    i += 1
                inst.sync_info = mybir.SyncInfo(
                    on_wait=[waits[-1]], on_update=list(si.on_update or [])
                )
                i += 1
    return nc


def _ntiles(T):
    out, off = [], 0
    while off < T:
        n = min(512, T - off)
        out.append((off, n))
        off += n
    return out


def _build(T_pad, F_pad):
    nc = bass.Bass()

    xT = nc.dram_tensor("xT", [D, T_pad], DT, kind="ExternalInput")
    w1T = nc.dram_tensor("w1T", [D, D], DT, kind="ExternalInput")
    b1 = nc.dram_tensor("b1", [128, KCH], F32, kind="ExternalInput")
    w2T = nc.dram_tensor("w2T", [D, D], DT, kind="ExternalInput")
    b2 = nc.dram_tensor("b2", [128, KCH], F32, kind="ExternalInput")
    xfT = nc.dram_tensor("xfT", [D, F_pad], DT, kind="ExternalInput")
    wf1T = nc.dram_tensor("wf1T", [D, D], DT, kind="ExternalInput")
    bf1 = nc.dram_tensor("bf1", [128, KCH], F32, kind="ExternalInput")
    wf2T = nc.dram_tensor("wf2T", [D, D], DT, kind="ExternalInput")
    bf2 = nc.dram_tensor("bf2", [128, KCH], F32, kind="ExternalInput")
    yT = nc.dram_tensor("yT", [D, T_pad], F32, kind="ExternalOutput")
    yfT = nc.dram_tensor("yfT", [D, F_pad], F32, kind="ExternalOutput")

    Relu = mybir.ActivationFunctionType.Relu
    Ident = mybir.ActivationFunctionType.Identity

    with tile.TileContext(nc) as tc:
        with tc.tile_pool(name="cp", bufs=1) as cp, \
             tc.tile_pool(name="xp", bufs=1) as xp, \
             tc.tile_pool(name="hp", bufs=1) as hp, \
             tc.tile_pool(name="wp", bufs=1) as wp, \
             tc.tile_pool(name="yp", bufs=3) as yp, \
             tc.tile_pool(name="pp", bufs=8, space="PSUM") as pp:

            biases = {}
            for name, dram in (("b1", b1), ("b2", b2),
                               ("bf1", bf1), ("bf2", bf2)):
                t = cp.tile([128, KCH], F32, tag=name, name=name)
                # gpsimd queue: keep the tiny bias loads off the Sync
                # descriptor stream that feeds the PE-critical pair loads
                nc.gpsimd.dma_start(out=t, in_=dram[:, :])
                biases[name] = t

            def alloc_kchunks(T, tagp, pool, dt=DT):
                return [pool.tile([128, T], dt, tag=f"{tagp}{k}",
                                  name=f"{tagp}{k}")
                        for k in range(KCH)]

            def load_kchunks(ts, dram):
                for k in range(KCH):
                    nc.sync.dma_start(out=ts[k],
                                      in_=dram[k * 128:(k + 1) * 128, :])

            nt = _ntiles(T_pad)
            ntf = _ntiles(F_pad)

            # x token-blocks: separate tiles per (k, block) so each block's
            # matmuls depend only on that block's DMA
            xs = [[xp.tile([128, n], DT, tag=f"x{k}b{bi}", name=f"x{k}b{bi}")
                   for bi, (off, n) in enumerate(nt)] for k in range(KCH)]
            # xf is not arrival-critical: load it as ONE big DMA (fewer
            # descriptors + semaphores); per-k views slice the big tile
            xfbig = xp.tile([128, KCH * F_pad], DT, tag="xf", name="xf")
            xfs = [[xfbig[:, k * F_pad + off:k * F_pad + off + n]
                    for (off, n) in ntf] for k in range(KCH)]
            ws1 = alloc_kchunks(D, "w1", wp)
            ws2 = alloc_kchunks(D, "w2", wp)
            wsf1 = alloc_kchunks(D, "w1", wp)  # reuses w1 slots after L1
            wsf2 = alloc_kchunks(D, "w2", wp)  # trickles in behind L2's last m-sweep

            # DMA order = consumption order:
            # (w1_k, x_k[block0]) pairs -> PE starts after the first pair;
            # then remaining token blocks (each enables a full 8-bank sweep);
            # then w2 (for L2), fallback inputs, fallback L2 weights.
            for k in range(KCH):
                nc.sync.dma_start(out=ws1[k],
                                  in_=w1T[k * 128:(k + 1) * 128, :])
                off, n = nt[0]
                nc.sync.dma_start(out=xs[k][0],
                                  in_=xT[k * 128:(k + 1) * 128, off:off + n])
            for bi, (off, n) in enumerate(nt[1:], start=1):
                for k in range(KCH):
                    nc.sync.dma_start(
                        out=xs[k][bi],
                        in_=xT[k * 128:(k + 1) * 128, off:off + n])
            load_kchunks(ws2, w2T)
            load_kchunks(wsf1, wf1T)
            nc.sync.dma_start(
                out=xfbig.rearrange("p (k t) -> p k t", k=KCH),
                in_=xfT.rearrange("(k p) t -> p k t", p=128))
            load_kchunks(wsf2, wf2T)

            def layer_blocked(src, ws, ntl, out_cb):
                """L1 form: block 0 gets a full 8-bank PSUM sweep with k
                inner (PE starts on the first arriving chunk pair); later
                blocks — data already resident by then — run weight-reuse
                style (m outer) so each LDWEIGHTS covers several matmuls."""
                bi0, (off0, n0) = 0, ntl[0]
                pss = [pp.tile([128, n0], F32, tag="ps", name="ps")
                       for _ in range(KCH)]
                for k in range(KCH):
                    for m in range(KCH):
                        nc.tensor.matmul(
                            pss[m],
                            ws[k][:, m * 128:(m + 1) * 128],
                            src[k][bi0],
                            start=(k == 0),
                            stop=(k == KCH - 1),
                        )
                for m in range(KCH):
                    out_cb(m, off0, n0, pss[m])
                if len(ntl) == 1:
                    return
                for m in range(KCH):
                    pss2 = {bi: pp.tile([128, n], F32, tag="ps", name="ps")
                            for bi, (off, n) in enumerate(ntl) if bi > 0}
                    for k in range(KCH):
                        for bi, (off, n) in enumerate(ntl):
                            if bi == 0:
                                continue
                            nc.tensor.matmul(
                                pss2[bi],
                                ws[k][:, m * 128:(m + 1) * 128],
                                src[k][bi],
                                start=(k == 0),
                                stop=(k == KCH - 1),
                            )
                    for bi, (off, n) in enumerate(ntl):
                        if bi > 0:
                            out_cb(m, off, n, pss2[bi])

            def layer_wreuse(src, ws, ntl, out_cb):
                """L2 form: m outer, k mid, block inner — each stationary
                weight tile loaded once per (m, k), reused across blocks."""
                for m in range(KCH):
                    pss = {off: pp.tile([128, n], F32, tag="ps", name="ps")
                           for off, n in ntl}
                    for k in range(KCH):
                        for off, n in ntl:
                            nc.tensor.matmul(
                                pss[off],
                                ws[k][:, m * 128:(m + 1) * 128],
                                src[k][:, off:off + n],
                                start=(k == 0),
                                stop=(k == KCH - 1),
                            )
                    for off, n in ntl:
                        out_cb(m, off, n, pss[off])

            def to_h(hs, b1t):
                def cb(m, off, n, ps):
                    nc.scalar.activation(hs[m][:, off:off + n], ps, Relu,
                                         bias=b1t[:, m:m + 1])
                return cb

            def to_y(ydram, b2t, ytag):
                def cb(m, off, n, ps):
                    yt = yp.tile([128, n], F32, tag=ytag, name=ytag)
                    nc.scalar.activation(yt, ps, Ident, bias=b2t[:, m:m + 1])
                    nc.sync.dma_start(
                        out=ydram[m * 128:(m + 1) * 128, off:off + n], in_=yt)
                return cb

            hs = alloc_kchunks(T_pad, "h", hp)
            hfs = alloc_kchunks(F_pad, "hf", hp)

            layer_blocked(xs, ws1, nt, to_h(hs, biases["b1"]))
            layer_wreuse(hs, ws2, nt, to_y(yT, biases["b2"], "y"))
            layer_blocked(xfs, wsf1, ntf, to_h(hfs, biases["bf1"]))
            layer_wreuse(hfs, wsf2, ntf, to_y(yfT, biases["bf2"], "yf"))

    _split_multi_waits(nc)
    return nc


_NC_CACHE = {}


def _get_nc(T_pad, F_pad):
    key = (T_pad, F_pad)
    if key not in _NC_CACHE:
        _NC_CACHE[key] = _build(T_pad, F_pad)
    return _NC_CACHE[key]


def _round_up(v, m):
    return ((v + m - 1) // m) * m


def kernel(x, W1, b1, W2, b2, Wf1, bf1, Wf2, bf2, routes, capacity,
           _trace=False):
    x = np.ascontiguousarray(np.asarray(x, dtype=np.float32))
    W1 = np.asarray(W1, dtype=np.float32)
    b1 = np.asarray(b1, dtype=np.float32)
    W2 = np.asarray(W2, dtype=np.float32)
    b2 = np.asarray(b2, dtype=np.float32)
    Wf1 = np.asarray(Wf1, dtype=np.float32)
    bf1 = np.asarray(bf1, dtype=np.float32)
    Wf2 = np.asarray(Wf2, dtype=np.float32)
    bf2 = np.asarray(bf2, dtype=np.float32)
    routes = np.asarray(routes)
    capacity = int(np.asarray(capacity))

    B, Dm = x.shape
    E = W1.shape[0]
    Kk = routes.shape[1]
    assert Dm == D and E == NCORES

    # --- routing: exact reference semantics (vectorized cumsum) ---
    e = routes.reshape(-1).astype(np.int64)
    valid = (e >= 0) & (e < E)
    e_safe = np.where(valid, e, 0)
    idx = np.arange(B * Kk)
    oh = np.zeros((B * Kk, E), dtype=np.int32)
    oh[idx[valid], e[valid]] = 1
    rank = np.cumsum(oh, axis=0) - oh
    rank_at = rank[idx, e_safe]
    accept_flat = valid & (rank_at < capacity)
    used = accept_flat.reshape(B, Kk).sum(1)

    # per-expert accepted assignment lists (flat order == reference order)
    tok_lists, fidx_lists, counts = [], [], []
    for el in range(E):
        fidx = np.nonzero(accept_flat & (e_safe == el))[0]
        fidx_lists.append(fidx)
        tok_lists.append(fidx // Kk)
        counts.append(len(fidx))
    T_pad = max(256, _round_up(max(counts), 256))
    src_flat = np.full(B * Kk, -1, dtype=np.int64)
    for el in range(E):
        src_flat[fidx_lists[el]] = el * T_pad + np.arange(counts[el])

    dropped = np.nonzero(used == 0)[0]
    F = len(dropped)
    Fc = max(1, -(-F // NCORES))
    F_pad = max(128, _round_up(Fc, 128))

    res = None

    def run_device():
        nc = _get_nc(T_pad, F_pad)

        def btile(v):
            return np.ascontiguousarray(v.reshape(KCH, 128).T)

        in_maps = []
        shared = {
            "wf1T": np.ascontiguousarray(Wf1.T),
            "bf1": btile(bf1),
            "wf2T": np.ascontiguousarray(Wf2.T),
            "bf2": btile(bf2),
        }
        for el in range(E):
            toks = tok_lists[el]
            tpad = np.zeros(T_pad, dtype=np.int64)
            tpad[:len(toks)] = toks
            lo, hi = el * Fc, min((el + 1) * Fc, F)
            fpad = np.zeros(F_pad, dtype=np.int64)
            if hi > lo:
                fpad[:hi - lo] = dropped[lo:hi]
            in_maps.append({
                "xT": np.ascontiguousarray(x[tpad].T),
                "xfT": np.ascontiguousarray(x[fpad].T),
                "w1T": np.ascontiguousarray(W1[el].T),
                "b1": btile(b1[el]),
                "w2T": np.ascontiguousarray(W2[el].T),
                "b2": btile(b2[el]),
                **shared,
            })

        r = run_bass_kernel_spmd(nc, in_maps, core_ids=list(range(NCORES)),
                                 trace=_trace)
        G = np.zeros((E * T_pad + 1, D), dtype=np.float32)
        for el in range(E):
            G[el * T_pad:(el + 1) * T_pad] = r.results[el]["yT"].T
        fb = None
        if F > 0:
            fb = np.empty((F, D), dtype=np.float32)
            for el in range(E):
                lo, hi = el * Fc, min((el + 1) * Fc, F)
                if hi > lo:
                    fb[lo:hi] = r.results[el]["yfT"].T[:hi - lo]
        return G, fb, r

    def run_numpy():
        G = np.zeros((E * T_pad + 1, D), dtype=np.float32)
        for el in range(E):
            toks = tok_lists[el]
            if len(toks):
                h = np.maximum(x[toks] @ W1[el].T + b1[el], 0.0)
                G[el * T_pad:el * T_pad + len(toks)] = h @ W2[el].T + b2[el]
        fb = None
        if F > 0:
            xd = x[dropped]
            fb = np.maximum(xd @ Wf1.T + bf1, 0.0) @ Wf2.T + bf2
        return G, fb, None

    # the Bass kernel covers the canonical problem sizes; anything odd
    # (or a device failure) falls back to exact numpy
    fits = (Dm == D and E == NCORES and W1.shape[1] == D and W1.shape[2] == D
            and T_pad <= 1536 and F_pad <= 1024)
    G = fb_rows = None
    if fits:
        try:
            G, fb_rows, res = run_device()
        except Exception:
            if _trace:
                raise
            G = None
    if G is None:
        G, fb_rows, res = run_numpy()

    # --- combine ---
    src = np.where(src_flat >= 0, src_flat, E * T_pad).reshape(B, Kk)
    summed = G[src].sum(axis=1)
    out = summed / np.maximum(used, 1.0).astype(np.float32)[:, None]
    if F > 0:
        out[dropped] = fb_rows

    if _trace:
        return out, res
    return out


# revision 24
# speedup vs baseline: 1.0246x; 1.0246x over previous
"""Capacity-MoE Trainium2 kernel (8 NeuronCores, expert-parallel).

Contract: kernel(**inputs) takes the FULL inputs of reference.setup_inputs()
and returns the FULL [B, D] float32 output.

Strategy
--------
Host: replicate the reference's capacity-aware routing (a plain cumsum over
the one-hot routes — no feedback loop), build per-expert accepted-token
lists, and shard expert e's tokens (<= capacity) to core e.  Tokens whose
every route overflowed ("dropped") are sharded contiguously across all 8
cores for the fallback MLP.  Device (per core): two dense 2-layer MLP
streams — the core's expert MLP over its [T_pad] gathered tokens and the
fallback MLP over its dropped-token chunk — all activations kept transposed
[D, T] so no on-device transposes are needed.  Host: gather per-assignment
outputs, average by accept count, and patch dropped rows with the fallback.

Matmuls run as float32r (TF32-like reduced-precision fp32, ~1.5e-4 relative
error per matmul, 4x the throughput of true fp32 on the PE array) with fp32
PSUM accumulation.
"""

import os
import sys

for _p in ("/opt/trn_rl_repo",):
    if _p not in sys.path and os.path.isdir(_p):
        sys.path.append(_p)

import numpy as np

import concourse.bass as bass
import concourse.tile as tile
from concourse import mybir
from concourse.bass_utils import run_bass_kernel_spmd

F32 = mybir.dt.float32
DT = mybir.dt.float32r  # matmul operand dtype

D = 1024
NCORES = 8
KCH = 8  # contraction chunks of 128 (D / 128)


# ---------------------------------------------------------------------------
# walrus in this environment rejects instructions with >1 sync wait; split
# extra waits onto same-engine NoOps inserted directly before the offender.
def _split_multi_waits(nc):
    ctr = 0
    for f in nc.m.functions:
        for bb in f.blocks:
            il = bb.instructions
            i = 0
            while i < len(il):
                inst = il[i]
                si = inst.sync_info
                if si is None or si.on_wait is None or len(si.on_wait) <= 1:
                    i += 1
                    continue
                waits = list(si.on_wait)
                for w in waits[:-1]:
                    ctr += 1
                    nop = mybir.InstNoOp(name=f"waitsplit-{ctr}")
                    nop.engine = inst.engine
                    nop.sync_info = mybir.SyncInfo(on_wait=[w], on_update=[])
                    il.insert(i, nop)
                    i += 1
                inst.sync_info = mybir.SyncInfo(
                    on_wait=[waits[-1]], on_update=list(si.on_update or [])
                )
                i += 1
    return nc


def _ntiles(T):
    out, off = [], 0
    while off < T:
        n = min(512, T - off)
        out.append((off, n))
        off += n
    return out


def _build(T_pad, F_pad):
    nc = bass.Bass()

    xT = nc.dram_tensor("xT", [D, T_pad], DT, kind="ExternalInput")
    w1T = nc.dram_tensor("w1T", [D, D], DT, kind="ExternalInput")
    b1 = nc.dram_tensor("b1", [128, KCH], F32, kind="ExternalInput")
    w2T = nc.dram_tensor("w2T", [D, D], DT, kind="ExternalInput")
    b2 = nc.dram_tensor("b2", [128, KCH], F32, kind="ExternalInput")
    xfT = nc.dram_tensor("xfT", [D, F_pad], DT, kind="ExternalInput")
    wf1T = nc.dram_tensor("wf1T", [D, D], DT, kind="ExternalInput")
    bf1 = nc.dram_tensor("bf1", [128, KCH], F32, kind="ExternalInput")
    wf2T = nc.dram_tensor("wf2T", [D, D], DT, kind="ExternalInput")
    bf2 = nc.dram_tensor("bf2", [128, KCH], F32, kind="ExternalInput")
    yT = nc.dram_tensor("yT", [D, T_pad], F32, kind="ExternalOutput")
    yfT = nc.dram_tensor("yfT", [D, F_pad], F32, kind="ExternalOutput")

    Relu = mybir.ActivationFunctionType.Relu
    Ident = mybir.ActivationFunctionType.Identity

    with tile.TileContext(nc) as tc:
        with tc.tile_pool(name="cp", bufs=1) as cp, \
             tc.tile_pool(name="xp", bufs=1) as xp, \
             tc.tile_pool(name="hp", bufs=1) as hp, \
             tc.tile_pool(name="wp", bufs=1) as wp, \
             tc.tile_pool(name="yp", bufs=3) as yp, \
             tc.tile_pool(name="pp", bufs=8, space="PSUM") as pp:

            biases = {}
            for name, dram in (("b1", b1), ("b2", b2),
                               ("bf1", bf1), ("bf2", bf2)):
                t = cp.tile([128, KCH], F32, tag=name, name=name)
                # gpsimd queue: keep the tiny bias loads off the Sync
                # descriptor stream that feeds the PE-critical pair loads
                nc.gpsimd.dma_start(out=t, in_=dram[:, :])
                biases[name] = t

            def alloc_kchunks(T, tagp, pool, dt=DT):
                return [pool.tile([128, T], dt, tag=f"{tagp}{k}",
                                  name=f"{tagp}{k}")
                        for k in range(KCH)]

            def load_kchunks(ts, dram):
                for k in range(KCH):
                    nc.sync.dma_start(out=ts[k],
                                      in_=dram[k * 128:(k + 1) * 128, :])

            nt = _ntiles(T_pad)
            ntf = _ntiles(F_pad)

            # x token-blocks: separate tiles per (k, block) so each block's
            # matmuls depend only on that block's DMA
            xs = [[xp.tile([128, n], DT, tag=f"x{k}b{bi}", name=f"x{k}b{bi}")
                   for bi, (off, n) in enumerate(nt)] for k in range(KCH)]
            xfs = [[xp.tile([128, n], DT, tag=f"x{k}b{bi}", name=f"xf{k}b{bi}")
                    for bi, (off, n) in enumerate(ntf)] for k in range(KCH)]
            ws1 = alloc_kchunks(D, "w1", wp)
            ws2 = alloc_kchunks(D, "w2", wp)
            wsf1 = alloc_kchunks(D, "w1", wp)  # reuses w1 slots after L1
            wsf2 = alloc_kchunks(D, "w2", wp)  # trickles in behind L2's last m-sweep

            # DMA order = consumption order:
            # (w1_k, x_k[block0]) pairs -> PE starts after the first pair;
            # then remaining token blocks (each enables a full 8-bank sweep);
            # then w2 (for L2), fallback inputs, fallback L2 weights.
            for k in range(KCH):
                nc.sync.dma_start(out=ws1[k],
                                  in_=w1T[k * 128:(k + 1) * 128, :])
                off, n = nt[0]
                nc.sync.dma_start(out=xs[k][0],
                                  in_=xT[k * 128:(k + 1) * 128, off:off + n])
            for bi, (off, n) in enumerate(nt[1:], start=1):
                for k in range(KCH):
                    nc.sync.dma_start(
                        out=xs[k][bi],
                        in_=xT[k * 128:(k + 1) * 128, off:off + n])
            load_kchunks(ws2, w2T)
            for k in range(KCH):
                nc.sync.dma_start(out=wsf1[k],
                                  in_=wf1T[k * 128:(k + 1) * 128, :])
                for bi, (off, n) in enumerate(ntf):
                    nc.sync.dma_start(
                        out=xfs[k][bi],
                        in_=xfT[k * 128:(k + 1) * 128, off:off + n])
            load_kchunks(wsf2, wf2T)

            def layer_blocked(src, ws, ntl, out_cb):
                """L1 form: block 0 gets a full 8-bank PSUM sweep with k
                inner (PE starts on the first arriving chunk pair); later
                blocks — data already resident by then — run weight-reuse
                style (m outer) so each LDWEIGHTS covers several matmuls."""
                bi0, (off0, n0) = 0, ntl[0]
                pss = [pp.tile([128, n0], F32, tag="ps", name="ps")
                       for _ in range(KCH)]
                for k in range(KCH):
                    for m in range(KCH):
                        nc.tensor.matmul(
                            pss[m],
                            ws[k][:, m * 128:(m + 1) * 128],
                            src[k][bi0],
                            start=(k == 0),
                            stop=(k == KCH - 1),
                        )
                for m in range(KCH):
                    out_cb(m, off0, n0, pss[m])
                if len(ntl) == 1:
                    return
                for m in range(KCH):
                    pss2 = {bi: pp.tile([128, n], F32, tag="ps", name="ps")
                            for bi, (off, n) in enumerate(ntl) if bi > 0}
                    for k in range(KCH):
                        for bi, (off, n) in enumerate(ntl):
                            if bi == 0:
                                continue
                            nc.tensor.matmul(
                                pss2[bi],
                                ws[k][:, m * 128:(m + 1) * 128],
                                src[k][bi],
                                start=(k == 0),
                                stop=(k == KCH - 1),
                            )
                    for bi, (off, n) in enumerate(ntl):
                        if bi > 0:
                            out_cb(m, off, n, pss2[bi])

            def layer_wreuse(src, ws, ntl, out_cb):
                """L2 form: m outer, k mid, block inner — each stationary
                weight tile loaded once per (m, k), reused across blocks."""
                for m in range(KCH):
                    pss = {off: pp.tile([128, n], F32, tag="ps", name="ps")
                           for off, n in ntl}
                    for k in range(KCH):
                        for off, n in ntl:
                            nc.tensor.matmul(
                                pss[off],
                                ws[k][:, m * 128:(m + 1) * 128],
                                src[k][:, off:off + n],
                                start=(k == 0),
                                stop=(k == KCH - 1),
                            )
                    for off, n in ntl:
                        out_cb(m, off, n, pss[off])

            def to_h(hs, b1t):
                def cb(m, off, n, ps):
                    nc.scalar.activation(hs[m][:, off:off + n], ps, Relu,
                                         bias=b1t[:, m:m + 1])
                return cb

            def to_y(ydram, b2t, ytag):
                def cb(m, off, n, ps):
                    yt = yp.tile([128, n], F32, tag=ytag, name=ytag)
                    nc.scalar.activation(yt, ps, Ident, bias=b2t[:, m:m + 1])
                    nc.sync.dma_start(
                        out=ydram[m * 128:(m + 1) * 128, off:off + n], in_=yt)
                return cb

            hs = alloc_kchunks(T_pad, "h", hp)
            hfs = alloc_kchunks(F_pad, "hf", hp)

            layer_blocked(xs, ws1, nt, to_h(hs, biases["b1"]))
            layer_wreuse(hs, ws2, nt, to_y(yT, biases["b2"], "y"))
            layer_blocked(xfs, wsf1, ntf, to_h(hfs, biases["bf1"]))
            layer_wreuse(hfs, wsf2, ntf, to_y(yfT, biases["bf2"], "yf"))

    _split_multi_waits(nc)
    return nc


_NC_CACHE = {}


def _get_nc(T_pad, F_pad):
    key = (T_pad, F_pad)
    if key not in _NC_CACHE:
        _NC_CACHE[key] = _build(T_pad, F_pad)
    return _NC_CACHE[key]


def _round_up(v, m):
    return ((v + m - 1) // m) * m


def kernel(x, W1, b1, W2, b2, Wf1, bf1, Wf2, bf2, routes, capacity,
           _trace=False):
    x = np.ascontiguousarray(np.asarray(x, dtype=np.float32))
    W1 = np.asarray(W1, dtype=np.float32)
    b1 = np.asarray(b1, dtype=np.float32)
    W2 = np.asarray(W2, dtype=np.float32)
    b2 = np.asarray(b2, dtype=np.float32)
    Wf1 = np.asarray(Wf1, dtype=np.float32)
    bf1 = np.asarray(bf1, dtype=np.float32)
    Wf2 = np.asarray(Wf2, dtype=np.float32)
    bf2 = np.asarray(bf2, dtype=np.float32)
    routes = np.asarray(routes)
    capacity = int(np.asarray(capacity))

    B, Dm = x.shape
    E = W1.shape[0]
    Kk = routes.shape[1]
    assert Dm == D and E == NCORES

    # --- routing: exact reference semantics (vectorized cumsum) ---
    e = routes.reshape(-1).astype(np.int64)
    valid = (e >= 0) & (e < E)
    e_safe = np.where(valid, e, 0)
    idx = np.arange(B * Kk)
    oh = np.zeros((B * Kk, E), dtype=np.int32)
    oh[idx[valid], e[valid]] = 1
    rank = np.cumsum(oh, axis=0) - oh
    rank_at = rank[idx, e_safe]
    accept_flat = valid & (rank_at < capacity)
    used = accept_flat.reshape(B, Kk).sum(1)

    # per-expert accepted assignment lists (flat order == reference order)
    tok_lists, fidx_lists, counts = [], [], []
    for el in range(E):
        fidx = np.nonzero(accept_flat & (e_safe == el))[0]
        fidx_lists.append(fidx)
        tok_lists.append(fidx // Kk)
        counts.append(len(fidx))
    T_pad = max(256, _round_up(max(counts), 256))
    src_flat = np.full(B * Kk, -1, dtype=np.int64)
    for el in range(E):
        src_flat[fidx_lists[el]] = el * T_pad + np.arange(counts[el])

    dropped = np.nonzero(used == 0)[0]
    F = len(dropped)
    Fc = max(1, -(-F // NCORES))
    F_pad = max(128, _round_up(Fc, 128))

    res = None

    def run_device():
        nc = _get_nc(T_pad, F_pad)

        def btile(v):
            return np.ascontiguousarray(v.reshape(KCH, 128).T)

        in_maps = []
        shared = {
            "wf1T": np.ascontiguousarray(Wf1.T),
            "bf1": btile(bf1),
            "wf2T": np.ascontiguousarray(Wf2.T),
            "bf2": btile(bf2),
        }
        for el in range(E):
            toks = tok_lists[el]
            tpad = np.zeros(T_pad, dtype=np.int64)
            tpad[:len(toks)] = toks
            lo, hi = el * Fc, min((el + 1) * Fc, F)
            fpad = np.zeros(F_pad, dtype=np.int64)
            if hi > lo:
                fpad[:hi - lo] = dropped[lo:hi]
            in_maps.append({
                "xT": np.ascontiguousarray(x[tpad].T),
                "xfT": np.ascontiguousarray(x[fpad].T),
                "w1T": np.ascontiguousarray(W1[el].T),
                "b1": btile(b1[el]),
                "w2T": np.ascontiguousarray(W2[el].T),
                "b2": btile(b2[el]),
                **shared,
            })

        r = run_bass_kernel_spmd(nc, in_maps, core_ids=list(range(NCORES)),
                                 trace=_trace)
        G = np.zeros((E * T_pad + 1, D), dtype=np.float32)
        for el in range(E):
            G[el * T_pad:(el + 1) * T_pad] = r.results[el]["yT"].T
        fb = None
        if F > 0:
            fb = np.empty((F, D), dtype=np.float32)
            for el in range(E):
                lo, hi = el * Fc, min((el + 1) * Fc, F)
                if hi > lo:
                    fb[lo:hi] = r.results[el]["yfT"].T[:hi - lo]
        return G, fb, r

    def run_numpy():
        G = np.zeros((E * T_pad + 1, D), dtype=np.float32)
        for el in range(E):
            toks = tok_lists[el]
            if len(toks):
                h = np.maximum(x[toks] @ W1[el].T + b1[el], 0.0)
                G[el * T_pad:el * T_pad + len(toks)] = h @ W2[el].T + b2[el]
        fb = None
        if F > 0:
            xd = x[dropped]
            fb = np.maximum(xd @ Wf1.T + bf1, 0.0) @ Wf2.T + bf2
        return G, fb, None

    # the Bass kernel covers the canonical problem sizes; anything odd
    # (or a device failure) falls back to exact numpy
    fits = (Dm == D and E == NCORES and W1.shape[1] == D and W1.shape[2] == D
            and T_pad <= 1536 and F_pad <= 1024)
    G = fb_rows = None
    if fits:
        try:
            G, fb_rows, res = run_device()
        except Exception:
            if _trace:
                raise
            G = None
    if G is None:
        G, fb_rows, res = run_numpy()

    # --- combine ---
    src = np.where(src_flat >= 0, src_flat, E * T_pad).reshape(B, Kk)
    summed = G[src].sum(axis=1)
    out = summed / np.maximum(used, 1.0).astype(np.float32)[:, None]
    if F > 0:
        out[dropped] = fb_rows

    if _trace:
        return out, res
    return out
